# revision 1
# baseline (speedup 1.0000x reference)
"""Multi-head dot-product attention (B=2, Q=K=2048, EMB=2048, H=32, D=64) on 8 TRN2 cores.

Sharding: data parallel over batch (2) x tensor parallel over heads (4 groups of 8).
Core c handles batch c//4, heads 8*(c%4) .. 8*(c%4)+8. Each core computes a partial
output [2048, 2048] (its heads' contribution through wo); host sums the 4 head-group
partials per batch.

On-chip dataflow (per core, all matmuls bf16, T=2048 tokens, HD=512=8 heads x 64):
  phase 1: q^T[hd,t]   = wq[e,hd].T @ xq^T[e,t]      (xq^T via DMA-transpose loads)
  phase 2: k^T[hd,t], v[k,hd] from xkv^T
  phase 3: per head: s^T[k,q] = k^T.T @ q^T (contract d=64, head pairs ride the
           PE row-groups 0-63/64-127 concurrently); attn = exp(s)*exp(bias)^T
           (stable softmax shift is unnecessary: |s+bias| <= ~8); ctx^T[d,q] and
           row-sums via ones-augmented v (M=65); normalize with approx reciprocal.
  phase 4: out[t,e] = ctx^T.T @ wo
"""

import numpy as np
import ml_dtypes
from contextlib import ExitStack

import concourse.bass as bass
from concourse import bacc
import concourse.mybir as mybir
import concourse.tile as tile
from concourse.bass_utils import run_bass_kernel_spmd

BF16 = mybir.dt.bfloat16
F32 = mybir.dt.float32
AF = mybir.ActivationFunctionType

B, T, E = 2, 2048, 2048          # batch, tokens (Q=K), embed
H, D = 32, 64                     # total heads, head dim
NH = 8                            # heads per core
HD = NH * D                       # 512, per-core head-dim total
EC = E // 128                     # 16 contraction chunks
KC = T // 128                     # 16 key chunks
QCH = 1024                        # attention q-chunk (psum-sized)
N_CORES = 8


def build_program(debug_dump=False):
    nc = bacc.Bacc("TRN2", target_bir_lowering=False, debug=False,
                   num_devices=N_CORES)

    xqT = nc.dram_tensor("xqT", [E, T], BF16, kind="ExternalInput").ap()
    xkvT = nc.dram_tensor("xkvT", [E, T], BF16, kind="ExternalInput").ap()
    biasT = nc.dram_tensor("biasT", [T, T], BF16, kind="ExternalInput").ap()
    wq = nc.dram_tensor("wq", [E, HD], BF16, kind="ExternalInput").ap()
    wk = nc.dram_tensor("wk", [E, HD], BF16, kind="ExternalInput").ap()
    wv = nc.dram_tensor("wv", [E, HD], BF16, kind="ExternalInput").ap()
    wo = nc.dram_tensor("wo", [HD, E], BF16, kind="ExternalInput").ap()
    out = nc.dram_tensor("out", [T, E], F32, kind="ExternalOutput").ap()
    dumps = {}
    if debug_dump:
        dumps["qT"] = nc.dram_tensor("d_qT", [128, HD // 128, T], F32, kind="ExternalOutput").ap()
        dumps["kT"] = nc.dram_tensor("d_kT", [128, HD // 128, T], F32, kind="ExternalOutput").ap()
        dumps["v"] = nc.dram_tensor("d_v", [128, KC, NH, D + 1], F32, kind="ExternalOutput").ap()
        dumps["expb"] = nc.dram_tensor("d_expb", [128, KC, T], F32, kind="ExternalOutput").ap()
        dumps["s00"] = nc.dram_tensor("d_s00", [128, QCH], F32, kind="ExternalOutput").ap()
        dumps["at00"] = nc.dram_tensor("d_at00", [128, QCH], F32, kind="ExternalOutput").ap()
        dumps["a200"] = nc.dram_tensor("d_a200", [128, QCH], F32, kind="ExternalOutput").ap()
        dumps["ctx0"] = nc.dram_tensor("d_ctx0", [D + 1, QCH], F32, kind="ExternalOutput").ap()
        dumps["rec0"] = nc.dram_tensor("d_rec0", [1, QCH], F32, kind="ExternalOutput").ap()
        dumps["recb0"] = nc.dram_tensor("d_recb0", [D, QCH], F32, kind="ExternalOutput").ap()
        dumps["ctxT"] = nc.dram_tensor("d_ctxT", [128, HD // 128, T], F32, kind="ExternalOutput").ap()

    with tile.TileContext(nc) as tc, ExitStack() as ctx:
        persist = ctx.enter_context(tc.tile_pool(name="persist", bufs=1))
        dbgpool = ctx.enter_context(tc.tile_pool(name="dbg", bufs=1)) if debug_dump else None

        def dump(key, ap):
            if not debug_dump:
                return
            if ap.space == bass.MemorySpace.PSUM:
                stage = dbgpool.tile(list(ap.shape), BF16, name=f"dbg_{key}")
                nc.vector.tensor_copy(stage[:], ap)
                nc.gpsimd.dma_start(out=dumps[key], in_=stage[:])
            else:
                nc.gpsimd.dma_start(out=dumps[key], in_=ap)  # casts bf16->f32

        qT_sb = persist.tile([128, HD // 128, T], BF16)   # q^T[hd, t], hd = hdc*128+p
        kT_sb = persist.tile([128, HD // 128, T], BF16)
        v_sb = persist.tile([128, KC, NH, D + 1], BF16)   # v[k, h, d] + ones col
        expb_sb = persist.tile([128, KC, T], BF16)        # exp(bias)^T[k, q]

        nc.vector.memset(v_sb[:, :, :, D:D + 1], 1.0)

        # weights for phases 1-2 (loads overlap phase-1 compute)
        wpool = tc.alloc_tile_pool(name="wpool", bufs=1)
        wk_sb = wpool.tile([128, EC, HD], BF16)
        wv_sb = wpool.tile([128, EC, HD], BF16)

        # ---- phase 1: q^T projection (xq^T streamed) + exp(bias^T) load ----
        p1pool = tc.alloc_tile_pool(name="p1", bufs=1)
        p12s = tc.alloc_tile_pool(name="p12s", bufs=2)
        rawpool = tc.alloc_tile_pool(name="p1raw", bufs=2)
        ps12 = tc.alloc_tile_pool(name="ps12", bufs=4, space="PSUM")
        if True:
            wq_sb = p1pool.tile([128, EC, HD], BF16)
            # chunked so the first matmuls start as soon as possible
            for eg in range(4):
                nc.sync.dma_start(
                    out=wq_sb[:, eg * 4:(eg + 1) * 4, :],
                    in_=wq[eg * 512:(eg + 1) * 512, :].rearrange("(ec p) n -> p ec n", p=128))
            for tc4 in range(4):
                if tc4 == 1:
                    # prefetches ride the SWDGE queue so they don't delay the
                    # critical xq stream chunks on the sync HWDGE queue
                    nc.gpsimd.dma_start(out=wk_sb[:],
                                        in_=wk.rearrange("(ec p) n -> p ec n", p=128))
                    nc.gpsimd.dma_start(out=wv_sb[:],
                                        in_=wv.rearrange("(ec p) n -> p ec n", p=128))
                if tc4 >= 1:
                    for kc in range((tc4 - 1) * 6, min(KC, tc4 * 6)):
                        raw = rawpool.tile([128, T], BF16)
                        nc.gpsimd.dma_start(out=raw[:], in_=biasT[kc * 128:(kc + 1) * 128, :])
                        nc.scalar.activation(expb_sb[:, kc, :], raw[:], AF.Exp)
                xqT_sb = p12s.tile([128, EC, 512], BF16, name="xs", tag="xs")
                nc.sync.dma_start(
                    out=xqT_sb[:],
                    in_=bass.AP(tensor=xqT.tensor, offset=xqT.offset + tc4 * 512,
                                ap=[[T, 128], [128 * T, EC], [1, 512]]))
                for hdc in range(HD // 128):
                    ps = ps12.tile([128, 512], F32, tag="pst")
                    for ec in range(EC):
                        nc.tensor.matmul(ps[:],
                                         lhsT=wq_sb[:, ec, hdc * 128:(hdc + 1) * 128],
                                         rhs=xqT_sb[:, ec, :],
                                         start=(ec == 0), stop=(ec == EC - 1))
                    nc.vector.tensor_copy(qT_sb[:, hdc, tc4 * 512:(tc4 + 1) * 512], ps[:])

        # ---- phase 2: k^T and v projections (xkv^T streamed by token chunk) ----
        if True:
            for tc4 in range(4):
                xkvT_sb = p12s.tile([128, EC, 512], BF16, name="xs", tag="xs")
                nc.sync.dma_start(
                    out=xkvT_sb[:],
                    in_=bass.AP(tensor=xkvT.tensor, offset=xkvT.offset + tc4 * 512,
                                ap=[[T, 128], [128 * T, EC], [1, 512]]))
                for hdc in range(HD // 128):
                    ps = ps12.tile([128, 512], F32, tag="pst")
                    for ec in range(EC):
                        nc.tensor.matmul(ps[:],
                                         lhsT=wk_sb[:, ec, hdc * 128:(hdc + 1) * 128],
                                         rhs=xkvT_sb[:, ec, :],
                                         start=(ec == 0), stop=(ec == EC - 1))
                    nc.vector.tensor_copy(kT_sb[:, hdc, tc4 * 512:(tc4 + 1) * 512], ps[:])
                for sub in range(4):
                    kc = tc4 * 4 + sub
                    ps = ps12.tile([128, 512], F32, tag="pst")
                    for ec in range(EC):
                        nc.tensor.matmul(ps[:],
                                         lhsT=xkvT_sb[:, ec, sub * 128:(sub + 1) * 128],
                                         rhs=wv_sb[:, ec, :],
                                         start=(ec == 0), stop=(ec == EC - 1))
                    nc.vector.tensor_copy(
                        v_sb[:, kc, :, 0:D],
                        ps.rearrange("p (h d) -> p h d", h=NH))

        if debug_dump:
            dump("qT", qT_sb[:])
            dump("kT", kT_sb[:])
            dump("v", v_sb[:])
            dump("expb", expb_sb[:])

        ps12.release()
        rawpool.release()
        p12s.release()
        p1pool.release()
        wpool.release()

        # pool for tensors that only live in phases 3-4 (reuses phase-1/2 space)
        late = ctx.enter_context(tc.tile_pool(name="late", bufs=1))
        ctxT_sb = late.tile([128, HD // 128, T], BF16)
        wo_sb = late.tile([128, HD // 128, E], BF16)
        nc.sync.dma_start(out=wo_sb[:], in_=wo.rearrange("(c p) n -> p c n", p=128))

        # ---- phase 3: attention (+ interleaved output projection) ----
        with tc.tile_pool(name="spsum", bufs=2, space="PSUM") as spsum, \
             tc.tile_pool(name="cpsum", bufs=2, space="PSUM") as cpsum, \
             tc.tile_pool(name="attn", bufs=3) as attnpool, \
             tc.tile_pool(name="attn2", bufs=3) as attnpool2, \
             tc.tile_pool(name="norm", bufs=2) as normpool, \
             tc.tile_pool(name="normd", bufs=2, space="DRAM") as normdram:
            for qc in range(T // QCH):
                for pair in range(NH // 2):
                    ctx_t = [cpsum.tile([D + 1, QCH], F32, tag="ctx", name=f"ctx{hh}")
                             for hh in range(2)]
                    for kc in range(KC):
                        attn2 = []
                        for hh in range(2):
                            h = pair * 2 + hh
                            pr = slice(hh * D, (hh + 1) * D)  # partition rows of this head
                            s = spsum.tile([128, QCH], F32, tag="s")
                            for half in range(QCH // 512):
                                q0 = qc * QCH + half * 512
                                nc.tensor.matmul(
                                    s[:, half * 512:(half + 1) * 512],
                                    lhsT=kT_sb[pr, pair, kc * 128:(kc + 1) * 128],
                                    rhs=qT_sb[pr, pair, q0:q0 + 512],
                                    start=True, stop=True)
                            at = attnpool.tile([128, QCH], BF16, tag="at")
                            nc.scalar.activation(at[:], s[:], AF.Exp)
                            a2 = attnpool2.tile([128, QCH], BF16, tag="a2")
                            nc.vector.tensor_mul(
                                a2[:], at[:], expb_sb[:, kc, qc * QCH:(qc + 1) * QCH])
                            attn2.append(a2)
                            if qc == 0 and pair == 0 and kc == 0 and hh == 0:
                                dump("s00", s[:])
                                dump("at00", at[:])
                                dump("a200", a2[:])
                        for hh in range(2):
                            h = pair * 2 + hh
                            for half in range(QCH // 512):
                                nc.tensor.matmul(
                                    ctx_t[hh][:, half * 512:(half + 1) * 512],
                                    lhsT=v_sb[:, kc, h, :],
                                    rhs=attn2[hh][:, half * 512:(half + 1) * 512],
                                    start=(kc == 0), stop=(kc == KC - 1))
                    if qc == 0 and pair == 0:
                        dump("ctx0", ctx_t[0][:])
                    for hh in range(2):
                        # engine ops must start at partition 0 on HW; move the
                        # sums row (psum partition 64) around with DMAs only
                        ctxf = normpool.tile([D + 1, QCH], F32, tag="ctxf", bufs=4)
                        nc.vector.tensor_copy(ctxf[:], ctx_t[hh][:])
                        srow = normpool.tile([1, QCH], F32, tag="srow")
                        nc.sync.dma_start(out=srow[:], in_=ctxf[D:D + 1, :])
                        rec = normpool.tile([1, QCH], F32, tag="rec")
                        nc.vector.reciprocal_approx_fast(out=rec[:], in_=srow[:])
                        rec_d = normdram.tile([QCH], F32, tag="recd")
                        nc.sync.dma_start(out=rec_d[:], in_=rec[:])
                        recb = normpool.tile([D, QCH], F32, tag="recb")
                        rd = rec_d[:]
                        bcast = bass.AP(tensor=rd.tensor, offset=rd.offset,
                                        ap=[[0, D]] + list(rd.ap))
                        nc.gpsimd.dma_start(out=recb[:], in_=bcast)
                        if qc == 0 and pair == 0 and hh == 0:
                            dump("rec0", rec[:])
                            dump("recb0", recb[:])
                        if hh == 0:
                            nc.vector.tensor_mul(
                                ctxT_sb[0:D, pair, qc * QCH:(qc + 1) * QCH],
                                ctxf[0:D, :], recb[:])
                        else:
                            # odd head belongs on partitions 64..127; DVE can't
                            # cross partitions, so normalize into a staging tile
                            # and DMA it into place
                            stage = normpool.tile([D, QCH], BF16, tag="cstage")
                            nc.vector.tensor_mul(stage[:], ctxf[0:D, :], recb[:])
                            nc.sync.dma_start(
                                out=ctxT_sb[D:2 * D, pair, qc * QCH:(qc + 1) * QCH],
                                in_=stage[:])

        if debug_dump:
            dump("ctxT", ctxT_sb[:])

        # ---- phase 4: output projection ----
        with tc.tile_pool(name="ps4", bufs=2, space="PSUM") as ps4, \
             tc.tile_pool(name="outp", bufs=2) as outpool:
            for tc16 in range(T // 128):
                po = ps4.tile([128, E], F32, tag="po")
                for hdc in range(HD // 128):
                    for ncol in range(E // 512):
                        nc.tensor.matmul(
                            po[:, ncol * 512:(ncol + 1) * 512],
                            lhsT=ctxT_sb[:, hdc, tc16 * 128:(tc16 + 1) * 128],
                            rhs=wo_sb[:, hdc, ncol * 512:(ncol + 1) * 512],
                            start=(hdc == 0), stop=(hdc == HD // 128 - 1),
                            skip_group_check=True)
                ot = outpool.tile([128, E], F32, tag="ot")
                nc.vector.tensor_copy(ot[:], po[:])
                nc.sync.dma_start(out[tc16 * 128:(tc16 + 1) * 128, :], ot[:])


    nc.compile()
    return nc


_NC_CACHE = {}


def kernel(inputs_q, inputs_kv, bias, wq, wk, wv, wo):
    bf16 = ml_dtypes.bfloat16
    inputs_q = np.asarray(inputs_q)
    inputs_kv = np.asarray(inputs_kv)
    bias = np.asarray(bias)
    # fold the reference's 1/sqrt(D) query scaling into wq
    wq_s = (np.asarray(wq).reshape(E, H * D) / np.sqrt(D)).astype(bf16)
    wk_s = np.asarray(wk).reshape(E, H * D).astype(bf16)
    wv_s = np.asarray(wv).reshape(E, H * D).astype(bf16)
    wo_s = np.asarray(wo).reshape(H * D, E).astype(bf16)

    # host-side layout marshaling: the kernel wants embed-major activations
    # and key-major bias (pure transposes, no math)
    xq_b = [np.ascontiguousarray(inputs_q[b].T).astype(bf16) for b in range(B)]
    xkv_b = [np.ascontiguousarray(inputs_kv[b].T).astype(bf16) for b in range(B)]
    bias_b = [np.ascontiguousarray(bias[b, 0].T).astype(bf16) for b in range(B)]

    in_maps = []
    for c in range(N_CORES):
        b, hg = c // 4, c % 4
        hs = slice(hg * HD, (hg + 1) * HD)
        in_maps.append({
            "xqT": xq_b[b],
            "xkvT": xkv_b[b],
            "biasT": bias_b[b],
            "wq": np.ascontiguousarray(wq_s[:, hs]),
            "wk": np.ascontiguousarray(wk_s[:, hs]),
            "wv": np.ascontiguousarray(wv_s[:, hs]),
            "wo": np.ascontiguousarray(wo_s[hs, :]),
        })

    if "nc" not in _NC_CACHE:
        _NC_CACHE["nc"] = build_program()
    nc = _NC_CACHE["nc"]

    res = run_bass_kernel_spmd(nc, in_maps, list(range(N_CORES)))
    outs = [np.asarray(r["out"], dtype=np.float32) for r in res.results]
    full = np.empty((B, T, E), dtype=np.float32)
    for b in range(B):
        full[b] = outs[4 * b] + outs[4 * b + 1] + outs[4 * b + 2] + outs[4 * b + 3]
    return full

